# revision 11
# baseline (speedup 1.0000x reference)
"""Trainium2 Bass kernel for nn_Decoder_33964601376783.

Key algorithmic fact: the reference resets x = decoderInput on every loop
iteration and overwrites `last`, so the output depends only on layer i=5.
We therefore compute ONE decoder layer (self-attn -> LN -> cross-attn -> LN
-> FFN -> LN) with layer-5 weights.

Sharding: 8-way sequence parallelism. Core c owns query rows
[c*256, (c+1)*256). K/V for self and cross attention are computed on the
owning core's sequence chunk and AllGathered (bf16). Everything on-device is
kept in "transposed" layout [feature, seq] so that matmul contractions always
have the contracted dim on partitions:

  Q^T/K^T [dk=128, s]      proj:    lhsT = W[dchunk, head-slice], rhs = x^T
  scores^T [t, s]          lhsT = K^T[:, tchunk], rhs = Q^T           (psum)
  A^T = exp(scores/1024)   ACT, bf16; causal handled via per-core exp bias
                           (-30 for future ranks) + fixed triangular masks
                           for the core's own diagonal chunk
  out^T [v, s] = V^T A^T   lhsT = V[tchunk, v-slice], rhs = A^T       (psum)
  denom [1, s]             lhsT = ones[128,1], rhs = A^T              (psum)
  LN over features = partition axis: sums via (1/1024)-matmuls, apply via
  row-vector -> [128, s] broadcast done with a K=1 matmul (ones ⊗ row).

Final output is produced transposed [1024, 256] per core; the host
transposes + concatenates (unshard).
"""

import sys

if "/opt/trn_rl_repo" not in sys.path:
    sys.path.insert(0, "/opt/trn_rl_repo")

import numpy as np
import ml_dtypes

NC = 8
SL = 256  # seq rows per core
D = 1024
H = 8
DKV = 128
FF = 4096
EPS = 1e-5
NEG = -30.0  # exp(-30) ~ 9e-14: masks future ranks in self-attention

BF16 = ml_dtypes.bfloat16

_prog_cache = {}


def _build_program(for_sim=False):
    import concourse.bass as bass
    import concourse.tile as tile
    from concourse import bacc, mybir

    dt = mybir.dt
    bf = dt.bfloat16
    f32 = dt.float32
    AF = mybir.ActivationFunctionType
    OP = mybir.AluOpType

    nc = bacc.Bacc("TRN2", target_bir_lowering=False, debug=False,
                   num_devices=(1 if for_sim else NC))

    def din(name, shape, dtype):
        return nc.dram_tensor(name, shape, dtype, kind="ExternalInput").ap()

    # --- per-core inputs ------------------------------------------------
    xT_bf = din("xT_bf", [D, SL], bf)
    xT_f32 = din("xT_f32", [D, SL], f32)
    eT_bf = din("eT_bf", [D, SL], bf)
    wq1 = din("wq1", [D, H * DKV], bf)
    wk1 = din("wk1", [D, H * DKV], bf)
    wv1 = din("wv1", [D, H * DKV], bf)
    wq2 = din("wq2", [D, H * DKV], bf)
    wk2 = din("wk2", [D, H * DKV], bf)
    wv2 = din("wv2", [D, H * DKV], bf)
    wf1 = din("wf1", [D, FF], bf)
    wf2 = din("wf2", [FF, D], bf)
    bq1 = din("bq1", [DKV, H], f32)
    bk1 = din("bk1", [DKV, H], f32)
    bv1 = din("bv1", [DKV, H], f32)
    bq2 = din("bq2", [DKV, H], f32)
    bk2 = din("bk2", [DKV, H], f32)
    bv2 = din("bv2", [DKV, H], f32)
    bf1 = din("bf1", [128, FF // 128], f32)
    bf2 = din("bf2", [128, D // 128], f32)
    gamT = din("gamT", [128, D // 128], f32)
    betT = din("betT", [128, D // 128], f32)
    tri0 = din("tri0", [128, SL], bf)
    tri1 = din("tri1", [128, SL], bf)
    bself = din("bself", [128, NC], f32)
    outT = nc.dram_tensor("outT", [D, SL], f32, kind="ExternalOutput").ap()

    with tile.TileContext(nc) as tc:
        with tc.tile_pool(name="const", bufs=1) as cpool, \
             tc.tile_pool(name="master", bufs=1) as mpool, \
             tc.tile_pool(name="dram", bufs=1, space="DRAM") as dpool:

            # ---- constants / small tiles in SBUF ----
            ones_bf = cpool.tile([128, 1], bf, name="ones_bf")
            nc.vector.memset(ones_bf[:], 1.0)
            inv1024 = cpool.tile([128, 1], bf, name="inv1024")
            nc.vector.memset(inv1024[:], 1.0 / 1024.0)
            ones_row = cpool.tile([1, 128], f32, name="ones_row")
            nc.vector.memset(ones_row[:], 1.0)
            eps_c = cpool.tile([1, 1], f32, name="eps_c")
            nc.vector.memset(eps_c[:], EPS)

            def sload(name, ap, shape, dtype):
                t = cpool.tile(shape, dtype, name=name)
                nc.sync.dma_start(t[:], ap)
                return t

            bq1s = sload("bq1s", bq1, [DKV, H], f32)
            bk1s = sload("bk1s", bk1, [DKV, H], f32)
            bv1s = sload("bv1s", bv1, [DKV, H], f32)
            bq2s = sload("bq2s", bq2, [DKV, H], f32)
            bk2s = sload("bk2s", bk2, [DKV, H], f32)
            bv2s = sload("bv2s", bv2, [DKV, H], f32)
            bf1s = sload("bf1s", bf1, [128, FF // 128], f32)
            bf2s = sload("bf2s", bf2, [128, D // 128], f32)
            gams = sload("gams", gamT, [128, D // 128], f32)
            bets = sload("bets", betT, [128, D // 128], f32)
            tris = [sload("tri0s", tri0, [128, SL], bf),
                    sload("tri1s", tri1, [128, SL], bf)]
            bselfs = sload("bselfs", bself, [128, NC], f32)

            # ---- activations in SBUF ----
            xbf = mpool.tile([128, 8, SL], bf, name="xbf")
            nc.sync.dma_start(xbf[:], xT_bf.rearrange("(c p) s -> p c s", p=128))
            x32 = mpool.tile([128, 8, SL], f32, name="x32")
            nc.sync.dma_start(x32[:], xT_f32.rearrange("(c p) s -> p c s", p=128))
            ebf = mpool.tile([128, 8, SL], bf, name="ebf")
            nc.sync.dma_start(ebf[:], eT_bf.rearrange("(c p) s -> p c s", p=128))

            # DRAM collective buffers. contrib layout: [2, 8, 128, 256]
            #   [0, h]       = K^T head h  [dk=128, t=256]
            #   [1, sub*4+q] = V[sub*128:(sub+1)*128, q*256:(q+1)*256] rows
            ag_space = "Local" if for_sim else "Shared"
            contrib_s = dpool.tile([2, 8, 128, SL], bf, name="contrib_s")
            ag_s = dpool.tile([NC, 2, 8, 128, SL], bf, name="ag_s",
                              addr_space=ag_space)
            contrib_c = dpool.tile([2, 8, 128, SL], bf, name="contrib_c")
            ag_c = dpool.tile([NC, 2, 8, 128, SL], bf, name="ag_c",
                              addr_space=ag_space)

            # masters that persist across phases (others allocated lazily
            # at first use with shared tags to bound SBUF)
            ktloc = mpool.tile([128, H, SL], bf, name="ktloc")   # self K^T local
            vloc = mpool.tile([128, 2, D], bf, name="vloc")      # self V local rows
            qts = mpool.tile([128, H, SL], bf, tag="qt", name="qts")

            # =========== Phase A: K/V projections + AllGathers ===========
            with tc.tile_pool(name="wkv", bufs=1) as wpool, \
                 tc.tile_pool(name="pproj", bufs=3, space="PSUM") as pproj, \
                 tc.tile_pool(name="tmps", bufs=3) as tpool:

                wk1s = wpool.tile([128, 8, H * DKV], bf, name="wk1s")
                nc.sync.dma_start(wk1s[:], wk1.rearrange("(c p) n -> p c n", p=128))
                wv1s = wpool.tile([128, 8, H * DKV], bf, name="wv1s")
                nc.sync.dma_start(wv1s[:], wv1.rearrange("(c p) n -> p c n", p=128))
                wk2s = wpool.tile([128, 8, H * DKV], bf, name="wk2s")
                nc.sync.dma_start(wk2s[:], wk2.rearrange("(c p) n -> p c n", p=128))
                wv2s = wpool.tile([128, 8, H * DKV], bf, name="wv2s")
                nc.sync.dma_start(wv2s[:], wv2.rearrange("(c p) n -> p c n", p=128))

                def proj_T(dst_ap, wt, rhs, h, bias_col):
                    ps = pproj.tile([128, SL], f32, tag="projps")
                    for dc in range(8):
                        nc.tensor.matmul(ps[:], wt[:, dc, h * 128:(h + 1) * 128],
                                         rhs[:, dc, :], start=(dc == 0),
                                         stop=(dc == 7))
                    if bias_col is not None:
                        nc.scalar.activation(dst_ap, ps[:], AF.Identity,
                                             bias=bias_col)
                    else:
                        nc.scalar.activation(dst_ap, ps[:], AF.Identity)

                def vrows(dst_tile, wt, rhs):
                    # V rows [t=2*128, h*v=1024] = x_chunk @ Wv_all
                    for sub in range(2):
                        for hf in range(2):
                            ps = pproj.tile([128, 512], f32, tag="vps")
                            for dc in range(8):
                                nc.tensor.matmul(
                                    ps[:],
                                    rhs[:, dc, sub * 128:(sub + 1) * 128],
                                    wt[:, dc, hf * 512:(hf + 1) * 512],
                                    start=(dc == 0), stop=(dc == 7))
                            nc.scalar.activation(
                                dst_tile[:, sub, hf * 512:(hf + 1) * 512],
                                ps[:], AF.Identity)

                # self K^T + V -> contrib_s -> AG1
                for h in range(H):
                    proj_T(ktloc[:, h, :], wk1s, xbf, h, bk1s[:, h:h + 1])
                nc.sync.dma_start(contrib_s[0].transpose([1, 0, 2]), ktloc[:])
                vrows(vloc, wv1s, xbf)
                nc.sync.dma_start(
                    contrib_s[1].rearrange("(u q) p t -> p u q t", u=2),
                    vloc[:].rearrange("p u (q t) -> p u q t", q=4))
                if for_sim:
                    for r in range(NC):
                        nc.sync.dma_start(ag_s[r], contrib_s[:])
                else:
                    nc.gpsimd.collective_compute(
                        "AllGather", OP.bypass,
                        replica_groups=[list(range(NC))],
                        ins=[contrib_s.opt()], outs=[ag_s.opt()])

                # cross K^T + V -> contrib_c -> AG2
                for h in range(H):
                    kc = tpool.tile([128, SL], bf, tag="kcross")
                    proj_T(kc[:], wk2s, ebf, h, bk2s[:, h:h + 1])
                    nc.sync.dma_start(contrib_c[0, h], kc[:])
                vc = tpool.tile([128, 2, D], bf, tag="vcross")
                vrows(vc, wv2s, ebf)
                nc.sync.dma_start(
                    contrib_c[1].rearrange("(u q) p t -> p u q t", u=2),
                    vc[:].rearrange("p u (q t) -> p u q t", q=4))
                if for_sim:
                    for r in range(NC):
                        nc.sync.dma_start(ag_c[r], contrib_c[:])
                else:
                    nc.gpsimd.collective_compute(
                        "AllGather", OP.bypass,
                        replica_groups=[list(range(NC))],
                        ins=[contrib_c.opt()], outs=[ag_c.opt()])

            # self Q^T (own pool so wq1 frees early)
            with tc.tile_pool(name="wq1p", bufs=1) as wq1pool, \
                 tc.tile_pool(name="pq", bufs=3, space="PSUM") as pq:
                wq1s = wq1pool.tile([128, 8, H * DKV], bf, name="wq1s")
                nc.sync.dma_start(wq1s[:], wq1.rearrange("(c p) n -> p c n", p=128))
                for h in range(H):
                    ps = pq.tile([128, SL], f32, tag="qps")
                    for dc in range(8):
                        nc.tensor.matmul(ps[:], wq1s[:, dc, h * 128:(h + 1) * 128],
                                         xbf[:, dc, :], start=(dc == 0),
                                         stop=(dc == 7))
                    nc.scalar.activation(qts[:, h, :], ps[:], AF.Identity,
                                         bias=bq1s[:, h:h + 1])

            # =========== attention + LN phases (shared pools) ===========
            def layer_norm(pool_sc, pool_rc, pool_tiny, tpool, pre32, dst32,
                           dstbf):
                prebf = tpool.tile([128, 8, SL], bf, tag="lnprebf")
                nc.scalar.activation(
                    prebf[:].rearrange("p c s -> p (c s)"),
                    pre32[:].rearrange("p c s -> p (c s)"), AF.Identity)
                stat = pool_tiny.tile([1, 2 * SL], f32, tag="tiny", name="stat")
                for dc in range(8):
                    nc.tensor.matmul(stat[:, 0:SL], inv1024[:],
                                     prebf[:, dc, :], start=(dc == 0),
                                     stop=(dc == 7))
                for dc in range(8):
                    sq = tpool.tile([128, SL], bf, tag="lnsq")
                    nc.vector.tensor_mul(sq[:], prebf[:, dc, :], prebf[:, dc, :])
                    nc.tensor.matmul(stat[:, SL:2 * SL], inv1024[:], sq[:],
                                     start=(dc == 0), stop=(dc == 7))
                stat_sb = tpool.tile([1, 2 * SL], f32, tag="lnstat_sb")
                nc.scalar.activation(stat_sb[:], stat[:], AF.Identity)
                mu = stat_sb[0:1, 0:SL]
                e2 = stat_sb[0:1, SL:2 * SL]
                musq = tpool.tile([1, SL], f32, tag="lnmusq")
                nc.vector.tensor_mul(musq[:], mu, mu)
                varv = tpool.tile([1, SL], f32, tag="lnvar")
                nc.vector.tensor_sub(varv[:], e2, musq[:])
                sd = tpool.tile([1, SL], f32, tag="lnsd")
                nc.scalar.activation(sd[:], varv[:], AF.Sqrt, bias=eps_c[:])
                rs = tpool.tile([1, SL], f32, tag="lnrs")
                nc.vector.reciprocal(rs[:], sd[:])
                b2 = tpool.tile([1, SL], f32, tag="lnb2")
                nc.vector.scalar_tensor_tensor(b2[:], mu, -1.0, rs[:],
                                               op0=OP.mult, op1=OP.mult)
                b1b = pool_rc.tile([128, SL], f32, tag="bcast")
                nc.tensor.matmul(b1b[:], ones_row[:],
                                 rs[:], start=True, stop=True)
                b2b = pool_rc.tile([128, SL], f32, tag="bcast")
                nc.tensor.matmul(b2b[:], ones_row[:],
                                 b2[:], start=True, stop=True)
                for dc in range(8):
                    t1 = tpool.tile([128, SL], f32, tag="lnt1")
                    nc.vector.tensor_mul(t1[:], pre32[:, dc, :], b1b[:])
                    t2 = tpool.tile([128, SL], f32, tag="lnt2")
                    nc.vector.tensor_add(t2[:], t1[:], b2b[:])
                    nc.vector.tensor_scalar(dst32[:, dc, :], t2[:],
                                            gams[:, dc:dc + 1],
                                            bets[:, dc:dc + 1],
                                            op0=OP.mult, op1=OP.add)
                if dstbf is not None:
                    nc.scalar.activation(
                        dstbf[:].rearrange("p c s -> p (c s)"),
                        dst32[:].rearrange("p c s -> p (c s)"), AF.Identity)

            with tc.tile_pool(name="psc", bufs=2, space="PSUM") as psc, \
                 tc.tile_pool(name="pav", bufs=2, space="PSUM") as pav, \
                 tc.tile_pool(name="prc", bufs=2, space="PSUM") as prc, \
                 tc.tile_pool(name="ptiny", bufs=2, space="PSUM") as ptiny, \
                 tc.tile_pool(name="vag", bufs=1) as vagpool, \
                 tc.tile_pool(name="kts", bufs=2) as ktspool, \
                 tc.tile_pool(name="lnt", bufs=2) as lnpool, \
                 tc.tile_pool(name="attn_t", bufs=4) as atpool:

                def attention(ag, qtile, vloc_tile, ktloc_tile, bias_cols,
                              bv_s, resid, dst_pre):
                    # V from AllGather: [128, j2=16, 1024], per-rank DMAs
                    vag = vagpool.tile([128, 16, D], bf, tag="vag")
                    for r in range(NC):
                        nc.sync.dma_start(
                            vag[:, r * 2:(r + 1) * 2, :].rearrange(
                                "p u (q t) -> p u q t", q=4),
                            ag[r, 1].rearrange("(u q) p t -> p u q t", u=2))
                    own = vloc_tile is not None
                    npos = 2 * NC + (2 if own else 0)
                    for h in range(H):
                        kts = ktspool.tile([128, NC, SL], bf, tag="kts")
                        nc.sync.dma_start(kts[:], ag[:, 0, h].transpose([1, 0, 2]))
                        avp = pav.tile([128, SL], f32, tag="avp")
                        dnp = ptiny.tile([1, SL], f32, tag="tiny", name="dnp")
                        pos = 0
                        for r in range(NC):
                            for sub in range(2):
                                scp = psc.tile([128, SL], f32, tag="scp")
                                nc.tensor.matmul(
                                    scp[:], kts[:, r, sub * 128:(sub + 1) * 128],
                                    qtile[:, h, :], start=True, stop=True)
                                at = atpool.tile([128, SL], bf, tag="at")
                                if bias_cols is not None:
                                    nc.scalar.activation(
                                        at[:], scp[:], AF.Exp,
                                        bias=bias_cols[:, r:r + 1],
                                        scale=1.0 / 1024.0)
                                else:
                                    nc.scalar.activation(at[:], scp[:], AF.Exp,
                                                         scale=1.0 / 1024.0)
                                nc.tensor.matmul(
                                    avp[:], vag[:, r * 2 + sub,
                                                h * 128:(h + 1) * 128],
                                    at[:], start=(pos == 0),
                                    stop=(pos == npos - 1))
                                nc.tensor.matmul(
                                    dnp[:], ones_bf[:], at[:],
                                    start=(pos == 0), stop=(pos == npos - 1))
                                pos += 1
                        if own:
                            for sub in range(2):
                                scp = psc.tile([128, SL], f32, tag="scp")
                                nc.tensor.matmul(
                                    scp[:],
                                    ktloc_tile[:, h, sub * 128:(sub + 1) * 128],
                                    qtile[:, h, :], start=True, stop=True)
                                at = atpool.tile([128, SL], bf, tag="at")
                                nc.scalar.activation(at[:], scp[:], AF.Exp,
                                                     scale=1.0 / 1024.0)
                                nc.vector.tensor_tensor(at[:], at[:],
                                                        tris[sub][:],
                                                        op=OP.mult)
                                nc.tensor.matmul(
                                    avp[:], vloc_tile[:, sub,
                                                      h * 128:(h + 1) * 128],
                                    at[:], start=(pos == 0),
                                    stop=(pos == npos - 1))
                                nc.tensor.matmul(
                                    dnp[:], ones_bf[:], at[:],
                                    start=(pos == 0), stop=(pos == npos - 1))
                                pos += 1
                        rc = atpool.tile([1, SL], f32, tag="rc")
                        nc.vector.reciprocal(rc[:], dnp[:])
                        rcb = prc.tile([128, SL], f32, tag="bcast")
                        nc.tensor.matmul(rcb[:], ones_row[:],
                                         rc[:], start=True,
                                         stop=True)
                        rcb_sb = atpool.tile([128, SL], f32, tag="rcb_sb")
                        nc.scalar.activation(rcb_sb[:], rcb[:], AF.Identity)
                        t1 = atpool.tile([128, SL], f32, tag="t1")
                        nc.vector.tensor_mul(t1[:], avp[:], rcb_sb[:])
                        nc.vector.scalar_tensor_tensor(
                            dst_pre[:, h, :], t1[:], bv_s[:, h:h + 1],
                            resid[:, h, :], op0=OP.add, op1=OP.add)

                # ---- self attention ----
                h1pre = mpool.tile([128, 8, SL], f32, tag="hpre", name="h1pre")
                attention(ag_s, qts, vloc, ktloc, bselfs, bv1s, x32, h1pre)
                h1 = mpool.tile([128, 8, SL], f32, tag="h", name="h1")
                h1bf = mpool.tile([128, 8, SL], bf, tag="hbf", name="h1bf")
                layer_norm(psc, prc, ptiny, lnpool, h1pre, h1, h1bf)

                # ---- cross Q projection (stream wq2 now) ----
                with tc.tile_pool(name="wq2p", bufs=1) as wq2pool:
                    wq2s = wq2pool.tile([128, 8, H * DKV], bf, name="wq2s")
                    nc.sync.dma_start(wq2s[:],
                                      wq2.rearrange("(c p) n -> p c n", p=128))
                    q2ts = mpool.tile([128, H, SL], bf, tag="qt", name="q2ts")
                    for h in range(H):
                        ps = psc.tile([128, SL], f32, tag="scp")
                        for dc in range(8):
                            nc.tensor.matmul(
                                ps[:], wq2s[:, dc, h * 128:(h + 1) * 128],
                                h1bf[:, dc, :], start=(dc == 0), stop=(dc == 7))
                        nc.scalar.activation(q2ts[:, h, :], ps[:], AF.Identity,
                                             bias=bq2s[:, h:h + 1])

                # ---- cross attention ----
                h2pre = mpool.tile([128, 8, SL], f32, tag="hpre", name="h2pre")
                attention(ag_c, q2ts, None, None, None, bv2s, h1, h2pre)
                h2 = mpool.tile([128, 8, SL], f32, tag="h", name="h2")
                h2bf = mpool.tile([128, 8, SL], bf, tag="hbf", name="h2bf")
                layer_norm(psc, prc, ptiny, lnpool, h2pre, h2, h2bf)

            # =========== FFN ===========
            zbf = mpool.tile([128, FF // 128, SL], bf, name="zbf")
            with tc.tile_pool(name="w1p", bufs=3) as w1pool, \
                 tc.tile_pool(name="pz", bufs=1, space="PSUM") as pz:
                for g in range(4):
                    zps = [pz.tile([128, SL], f32, tag=f"zps{ff}",
                                   name=f"zps{g}_{ff}")
                           for ff in range(8)]
                    for dc in range(8):
                        w1t = w1pool.tile([128, 1024], bf, tag="w1t")
                        nc.sync.dma_start(
                            w1t[:], wf1[dc * 128:(dc + 1) * 128,
                                        g * 1024:(g + 1) * 1024])
                        for ff in range(8):
                            nc.tensor.matmul(zps[ff][:],
                                             w1t[:, ff * 128:(ff + 1) * 128],
                                             h2bf[:, dc, :], start=(dc == 0),
                                             stop=(dc == 7))
                    for ff in range(8):
                        fi = g * 8 + ff
                        nc.scalar.activation(zbf[:, fi, :], zps[ff][:], AF.Relu,
                                             bias=bf1s[:, fi:fi + 1])

            h3pre = mpool.tile([128, 8, SL], f32, tag="hpre", name="h3pre")
            with tc.tile_pool(name="w2p", bufs=3) as w2pool, \
                 tc.tile_pool(name="po", bufs=1, space="PSUM") as po:
                ops = [po.tile([128, SL], f32, tag=f"ops{dc}",
                               name=f"ops{dc}")
                       for dc in range(8)]
                for ffc in range(32):
                    w2t = w2pool.tile([128, 1024], bf, tag="w2t")
                    nc.sync.dma_start(w2t[:],
                                      wf2[ffc * 128:(ffc + 1) * 128, :])
                    for dc in range(8):
                        nc.tensor.matmul(ops[dc][:],
                                         w2t[:, dc * 128:(dc + 1) * 128],
                                         zbf[:, ffc, :], start=(ffc == 0),
                                         stop=(ffc == 31))
                for dc in range(8):
                    nc.vector.scalar_tensor_tensor(
                        h3pre[:, dc, :], ops[dc][:], bf2s[:, dc:dc + 1],
                        h2[:, dc, :], op0=OP.add, op1=OP.add)

            out32 = mpool.tile([128, 8, SL], f32, tag="h", name="out32")
            with tc.tile_pool(name="pln3a", bufs=2, space="PSUM") as pa3, \
                 tc.tile_pool(name="pln3b", bufs=2, space="PSUM") as pb3, \
                 tc.tile_pool(name="ln3t", bufs=2) as lt3:
                layer_norm(pa3, pa3, pb3, lt3, h3pre, out32, None)

            nc.sync.dma_start(outT.rearrange("(c p) s -> p c s", p=128),
                              out32[:])

    nc.compile()
    return nc


def _get_program():
    if "nc" not in _prog_cache:
        _prog_cache["nc"] = _build_program()
    return _prog_cache["nc"]


def _prep_inputs(inputs):
    i = 5  # only the last layer matters (see module docstring)
    f32 = np.float32

    x = np.asarray(inputs["decoderInput"], f32)
    e = np.asarray(inputs["encoderOutput"], f32)

    def wcat(w):  # [H, D, dk] -> [D, H*dk]
        w = np.asarray(w, f32)
        return np.ascontiguousarray(w.transpose(1, 0, 2).reshape(D, H * DKV))

    shared = {
        "wq1": wcat(inputs["Wq1"][i]).astype(BF16),
        "wk1": wcat(inputs["Wk1"][i]).astype(BF16),
        "wv1": wcat(inputs["Wv1"][i]).astype(BF16),
        "wq2": wcat(inputs["Wq2"][i]).astype(BF16),
        "wk2": wcat(inputs["Wk2"][i]).astype(BF16),
        "wv2": wcat(inputs["Wv2"][i]).astype(BF16),
        "wf1": np.ascontiguousarray(np.asarray(inputs["Wff1"][i], f32)).astype(BF16),
        "wf2": np.ascontiguousarray(np.asarray(inputs["Wff2"][i], f32)).astype(BF16),
        "bq1": np.ascontiguousarray(np.asarray(inputs["bq1"][i], f32).T),
        "bk1": np.ascontiguousarray(np.asarray(inputs["bk1"][i], f32).T),
        "bv1": np.ascontiguousarray(np.asarray(inputs["bv1"][i], f32).T),
        "bq2": np.ascontiguousarray(np.asarray(inputs["bq2"][i], f32).T),
        "bk2": np.ascontiguousarray(np.asarray(inputs["bk2"][i], f32).T),
        "bv2": np.ascontiguousarray(np.asarray(inputs["bv2"][i], f32).T),
        "bf1": np.ascontiguousarray(
            np.asarray(inputs["bff1"][i], f32).reshape(FF // 128, 128).T),
        "bf2": np.ascontiguousarray(
            np.asarray(inputs["bff2"][i], f32).reshape(D // 128, 128).T),
        "gamT": np.ascontiguousarray(
            np.asarray(inputs["gamma"], f32).reshape(D // 128, 128).T),
        "betT": np.ascontiguousarray(
            np.asarray(inputs["beta"], f32).reshape(D // 128, 128).T),
    }
    tt, ss = np.arange(128)[:, None], np.arange(SL)[None, :]
    shared["tri0"] = (ss >= tt).astype(BF16)
    shared["tri1"] = (ss >= 128 + tt).astype(BF16)

    in_maps = []
    for c in range(NC):
        xc = x[c * SL:(c + 1) * SL, :].T  # [D, SL]
        ec = e[c * SL:(c + 1) * SL, :].T
        bs = np.zeros((128, NC), f32)
        bs[:, c:] = NEG  # ranks >= c masked in AG path (own handled locally)
        m = dict(shared)
        m["xT_bf"] = np.ascontiguousarray(xc).astype(BF16)
        m["xT_f32"] = np.ascontiguousarray(xc)
        m["eT_bf"] = np.ascontiguousarray(ec).astype(BF16)
        m["bself"] = bs
        in_maps.append(m)
    return in_maps


def _run(inputs, trace=False):
    from concourse.bass_utils import run_bass_kernel_spmd

    nc = _get_program()
    in_maps = _prep_inputs(inputs)
    res = run_bass_kernel_spmd(nc, in_maps, core_ids=list(range(NC)),
                               trace=trace)
    out = np.concatenate(
        [np.asarray(res.results[c]["outT"], np.float32).T for c in range(NC)],
        axis=0)
    return out, res


def kernel(**inputs) -> np.ndarray:
    out, _ = _run(inputs, trace=False)
    return out


# revision 20
# speedup vs baseline: 1.5189x; 1.5189x over previous
"""Trainium2 Bass kernel for nn_Decoder_33964601376783.

Key algorithmic fact: the reference resets x = decoderInput on every loop
iteration and overwrites `last`, so the output depends only on layer i=5.
We therefore compute ONE decoder layer (self-attn -> LN -> cross-attn -> LN
-> FFN -> LN) with layer-5 weights.

Sharding: 8-way sequence parallelism. Core c owns query rows
[c*256, (c+1)*256). K/V for self and cross attention are computed on the
owning core's sequence chunk and AllGathered (bf16). Everything on-device is
kept in "transposed" layout [feature, seq] so that matmul contractions always
have the contracted dim on partitions:

  Q^T/K^T [dk=128, s]      proj:    lhsT = W[dchunk, head-slice], rhs = x^T
  scores^T [t, s]          lhsT = K^T[:, tchunk], rhs = Q^T           (psum)
  A^T = exp(scores/1024)   ACT, bf16; causal handled via per-core exp bias
                           (-30 for future ranks) + fixed triangular masks
                           for the core's own diagonal chunk
  out^T [v, s] = V^T A^T   lhsT = V[tchunk, v-slice], rhs = A^T       (psum)
  denom [1, s]             lhsT = ones[128,1], rhs = A^T              (psum)
  LN over features = partition axis: sums via (1/1024)-matmuls, apply via
  row-vector -> [128, s] broadcast done with a K=1 matmul (ones ⊗ row).

Final output is produced transposed [1024, 256] per core; the host
transposes + concatenates (unshard).
"""

import sys

if "/opt/trn_rl_repo" not in sys.path:
    sys.path.insert(0, "/opt/trn_rl_repo")

import numpy as np
import ml_dtypes

NC = 8
SL = 256  # seq rows per core
D = 1024
H = 8
DKV = 128
FF = 4096
EPS = 1e-5
NEG = -30.0  # exp(-30) ~ 9e-14: masks future ranks in self-attention

BF16 = ml_dtypes.bfloat16

_prog_cache = {}


def _build_program(for_sim=False, phase_limit=99):
    import concourse.bass as bass
    import concourse.tile as tile
    from concourse import bacc, mybir

    dt = mybir.dt
    bf = dt.bfloat16
    f32 = dt.float32
    AF = mybir.ActivationFunctionType
    OP = mybir.AluOpType

    nc = bacc.Bacc("TRN2", target_bir_lowering=False, debug=False,
                   num_devices=(1 if for_sim else NC))

    def din(name, shape, dtype):
        return nc.dram_tensor(name, shape, dtype, kind="ExternalInput").ap()

    # --- per-core inputs ------------------------------------------------
    xT_bf = din("xT_bf", [D, SL], bf)
    xT_f32 = din("xT_f32", [D, SL], f32)
    eT_bf = din("eT_bf", [D, SL], bf)
    wq1 = din("wq1", [D, H * DKV], bf)
    wk1 = din("wk1", [D, H * DKV], bf)
    wv1 = din("wv1", [D, H * DKV], bf)
    wq2 = din("wq2", [D, H * DKV], bf)
    wk2 = din("wk2", [D, H * DKV], bf)
    wv2 = din("wv2", [D, H * DKV], bf)
    wf1 = din("wf1", [D, FF], bf)
    wf2 = din("wf2", [FF, D], bf)
    ball = din("ball", [128, 112], f32)
    tric = din("tric", [128, 2 * SL], bf)
    maskS = din("maskS", [128, 4096], bf)
    outT = nc.dram_tensor("outT", [D, SL], f32, kind="ExternalOutput").ap()

    with tile.TileContext(nc) as tc:
        with tc.tile_pool(name="const", bufs=1) as cpool, \
             tc.tile_pool(name="master", bufs=1) as mpool, \
             tc.tile_pool(name="dram", bufs=1, space="DRAM") as dpool:

            # ---- constants / small tiles in SBUF ----
            ones_bf = cpool.tile([128, 1], bf, name="ones_bf")
            nc.vector.memset(ones_bf[:], 1.0)
            inv1024 = cpool.tile([128, 1], bf, name="inv1024")
            nc.vector.memset(inv1024[:], 1.0 / 1024.0)
            ones_row = cpool.tile([1, 128], f32, name="ones_row")
            nc.vector.memset(ones_row[:], 1.0)
            eps_c = cpool.tile([1, 1], f32, name="eps_c")
            nc.vector.memset(eps_c[:], EPS)

            def sload(name, ap, shape, dtype):
                t = cpool.tile(shape, dtype, name=name)
                nc.sync.dma_start(t[:], ap)
                return t

            balls = sload("balls", ball, [128, 112], f32)
            bq1s = balls[:, 0:8]
            bk1s = balls[:, 8:16]
            bv1s = balls[:, 16:24]
            bq2s = balls[:, 24:32]
            bk2s = balls[:, 32:40]
            bv2s = balls[:, 40:48]
            bf1s = balls[:, 48:80]
            bf2s = balls[:, 80:88]
            gams = balls[:, 88:96]
            bets = balls[:, 96:104]
            trics = sload("trics", tric, [128, 2 * SL], bf)
            maskSs = sload("maskSs", maskS, [128, 4096], bf)

            # ---- activations in SBUF ----
            xbf = mpool.tile([128, 8, SL], bf, name="xbf")
            nc.sync.dma_start(xbf[:], xT_bf.rearrange("(c p) s -> p c s", p=128))
            x32 = mpool.tile([128, 8, SL], f32, name="x32")
            nc.sync.dma_start(x32[:], xT_f32.rearrange("(c p) s -> p c s", p=128))
            ebf = mpool.tile([128, 8, SL], bf, name="ebf")
            nc.sync.dma_start(ebf[:], eT_bf.rearrange("(c p) s -> p c s", p=128))

            # DRAM collective buffers. contrib layout: [2, 8, 128, 256]
            #   [0, h]       = K^T head h  [dk=128, t=256]
            #   [1, sub*4+q] = V[sub*128:(sub+1)*128, q*256:(q+1)*256] rows
            ag_space = "Local" if for_sim else "Shared"
            contrib_s = dpool.tile([2, 128, H, SL], bf, name="contrib_s")
            ag_s = dpool.tile([NC, 2, 128, H, SL], bf, name="ag_s",
                              addr_space=ag_space)
            contrib_c = dpool.tile([2, 128, H, SL], bf, name="contrib_c")
            ag_c = dpool.tile([NC, 2, 128, H, SL], bf, name="ag_c",
                              addr_space=ag_space)

            # masters that persist across phases (others allocated lazily
            # at first use with shared tags to bound SBUF)
            ktloc = mpool.tile([128, H, SL], bf, name="ktloc")   # self K^T local
            vloc = mpool.tile([128, 2, D], bf, name="vloc")      # self V local rows
            qts = mpool.tile([128, H, SL], bf, tag="qt", name="qts")

            # =========== Phase A: K/V projections + AllGathers ===========
            with tc.tile_pool(name="wkv", bufs=1) as wpool, \
                 tc.tile_pool(name="pproj", bufs=3, space="PSUM") as pproj, \
                 tc.tile_pool(name="tmps", bufs=3) as tpool:

                wk1s = wpool.tile([128, 8, H * DKV], bf, name="wk1s")
                nc.sync.dma_start(wk1s[:], wk1.rearrange("(c p) n -> p c n", p=128))
                wv1s = wpool.tile([128, 8, H * DKV], bf, name="wv1s")
                nc.sync.dma_start(wv1s[:], wv1.rearrange("(c p) n -> p c n", p=128))
                wk2s = wpool.tile([128, 8, H * DKV], bf, name="wk2s")
                nc.sync.dma_start(wk2s[:], wk2.rearrange("(c p) n -> p c n", p=128))
                wv2s = wpool.tile([128, 8, H * DKV], bf, name="wv2s")
                nc.sync.dma_start(wv2s[:], wv2.rearrange("(c p) n -> p c n", p=128))

                def proj_T(dst_ap, wt, rhs, h, bias_col):
                    ps = pproj.tile([128, SL], f32, tag="projps")
                    for dc in range(8):
                        nc.tensor.matmul(ps[:], wt[:, dc, h * 128:(h + 1) * 128],
                                         rhs[:, dc, :], start=(dc == 0),
                                         stop=(dc == 7))
                    if bias_col is not None:
                        nc.vector.tensor_scalar(dst_ap, ps[:], bias_col, None,
                                                op0=OP.add)
                    else:
                        nc.vector.tensor_copy(dst_ap, ps[:])

                def vrows(dst_tile, wt, rhs):
                    # V rows [t=2*128, h*v=1024] = x_chunk @ Wv_all
                    for sub in range(2):
                        for hf in range(2):
                            ps = pproj.tile([128, 512], f32, tag="vps")
                            for dc in range(8):
                                nc.tensor.matmul(
                                    ps[:],
                                    rhs[:, dc, sub * 128:(sub + 1) * 128],
                                    wt[:, dc, hf * 512:(hf + 1) * 512],
                                    start=(dc == 0), stop=(dc == 7))
                            nc.vector.tensor_copy(
                                dst_tile[:, sub, hf * 512:(hf + 1) * 512],
                                ps[:])

                # self K^T + V -> contrib_s -> AG1
                for h in range(H):
                    proj_T(ktloc[:, h, :], wk1s, xbf, h, bk1s[:, h:h + 1])
                nc.sync.dma_start(contrib_s[0], ktloc[:])
                vrows(vloc, wv1s, xbf)
                nc.sync.dma_start(
                    contrib_s[1].rearrange("p e t -> p (e t)"),
                    vloc[:].rearrange("p u n -> p (u n)"))
                if for_sim:
                    for r in range(NC):
                        nc.sync.dma_start(ag_s[r], contrib_s[:])
                else:
                    nc.gpsimd.collective_compute(
                        "AllGather", OP.bypass,
                        replica_groups=[list(range(NC))],
                        ins=[contrib_s.opt()], outs=[ag_s.opt()])

                # cross K^T + V -> contrib_c -> AG2
                ktc = tpool.tile([128, H, SL], bf, tag="ktcross")
                for h in range(H):
                    proj_T(ktc[:, h, :], wk2s, ebf, h, bk2s[:, h:h + 1])
                nc.sync.dma_start(contrib_c[0], ktc[:])
                vc = tpool.tile([128, 2, D], bf, tag="vcross")
                vrows(vc, wv2s, ebf)
                nc.sync.dma_start(
                    contrib_c[1].rearrange("p e t -> p (e t)"),
                    vc[:].rearrange("p u n -> p (u n)"))
                if for_sim:
                    for r in range(NC):
                        nc.sync.dma_start(ag_c[r], contrib_c[:])
                else:
                    nc.gpsimd.collective_compute(
                        "AllGather", OP.bypass,
                        replica_groups=[list(range(NC))],
                        ins=[contrib_c.opt()], outs=[ag_c.opt()])

            # self Q^T (own pool so wq1 frees early)
            with tc.tile_pool(name="wq1p", bufs=1) as wq1pool, \
                 tc.tile_pool(name="pq", bufs=3, space="PSUM") as pq:
                wq1s = wq1pool.tile([128, 8, H * DKV], bf, name="wq1s")
                nc.sync.dma_start(wq1s[:], wq1.rearrange("(c p) n -> p c n", p=128))
                for h in range(H):
                    ps = pq.tile([128, SL], f32, tag="qps")
                    for dc in range(8):
                        nc.tensor.matmul(ps[:], wq1s[:, dc, h * 128:(h + 1) * 128],
                                         xbf[:, dc, :], start=(dc == 0),
                                         stop=(dc == 7))
                    nc.vector.tensor_scalar(qts[:, h, :], ps[:],
                                            bq1s[:, h:h + 1], None, op0=OP.add)

            # =========== attention + LN phases (shared pools) ===========
            def layer_norm(pool_sc, pool_rc, tpool, pre32, dst32,
                           dstbf):
                prebf = tpool.tile([128, 8, SL], bf, tag="lnprebf")
                nc.vector.tensor_copy(
                    prebf[:].rearrange("p c s -> p (c s)"),
                    pre32[:].rearrange("p c s -> p (c s)"))
                stat = pool_rc.tile([1, 2 * SL], f32, tag="bcast", name="stat")
                for dc in range(8):
                    nc.tensor.matmul(stat[:, 0:SL], inv1024[:],
                                     prebf[:, dc, :], start=(dc == 0),
                                     stop=(dc == 7))
                for dc in range(8):
                    sq = tpool.tile([128, SL], bf, tag="lnsq")
                    nc.vector.tensor_mul(sq[:], prebf[:, dc, :], prebf[:, dc, :])
                    nc.tensor.matmul(stat[:, SL:2 * SL], inv1024[:], sq[:],
                                     start=(dc == 0), stop=(dc == 7))
                stat_sb = tpool.tile([1, 2 * SL], f32, tag="lnstat_sb")
                nc.vector.tensor_copy(stat_sb[:], stat[:])
                mu = stat_sb[0:1, 0:SL]
                e2 = stat_sb[0:1, SL:2 * SL]
                musq = tpool.tile([1, SL], f32, tag="lnmusq")
                nc.vector.tensor_mul(musq[:], mu, mu)
                varv = tpool.tile([1, SL], f32, tag="lnvar")
                nc.vector.tensor_sub(varv[:], e2, musq[:])
                sd = tpool.tile([1, SL], f32, tag="lnsd")
                nc.scalar.activation(sd[:], varv[:], AF.Sqrt, bias=eps_c[:])
                rs = tpool.tile([1, SL], f32, tag="lnrs")
                nc.vector.reciprocal(rs[:], sd[:])
                b2 = tpool.tile([1, SL], f32, tag="lnb2")
                nc.vector.scalar_tensor_tensor(b2[:], mu, -1.0, rs[:],
                                               op0=OP.mult, op1=OP.mult)
                bb = pool_rc.tile([128, 2 * SL], f32, tag="bcast")
                b1b = bb[:, 0:SL]
                b2b = bb[:, SL:2 * SL]
                nc.tensor.matmul(b1b, ones_row[:],
                                 rs[:], start=True, stop=True)
                nc.tensor.matmul(b2b, ones_row[:],
                                 b2[:], start=True, stop=True)
                for dc in range(8):
                    t1 = tpool.tile([128, SL], f32, tag="lnt1")
                    nc.vector.tensor_mul(t1[:], pre32[:, dc, :], b1b)
                    t2 = tpool.tile([128, SL], f32, tag="lnt2")
                    nc.vector.tensor_add(t2[:], t1[:], b2b)
                    nc.vector.tensor_scalar(dst32[:, dc, :], t2[:],
                                            gams[:, dc:dc + 1],
                                            bets[:, dc:dc + 1],
                                            op0=OP.mult, op1=OP.add)
                if dstbf is not None:
                    nc.vector.tensor_copy(
                        dstbf[:].rearrange("p c s -> p (c s)"),
                        dst32[:].rearrange("p c s -> p (c s)"))

            with tc.tile_pool(name="psc", bufs=2, space="PSUM") as psc, \
                 tc.tile_pool(name="pav", bufs=2, space="PSUM") as pav, \
                 tc.tile_pool(name="prc", bufs=2, space="PSUM") as prc, \
                 tc.tile_pool(name="vag", bufs=1) as vagpool, \
                 tc.tile_pool(name="kts", bufs=1) as ktspool, \
                 tc.tile_pool(name="lnt", bufs=2) as lnpool, \
                 tc.tile_pool(name="attn_t", bufs=4) as atpool:

                def attention(ag, qtile, vloc_tile, ktloc_tile, bias_cols,
                              bv_s, resid, dst_pre):
                    # V from AllGather: [128, j2=16, 1024], per-rank DMAs
                    vag = vagpool.tile([128, 16, D], bf, tag="vag")
                    nc.sync.dma_start(
                        vag[:].rearrange("p j n -> p (j n)").rearrange(
                            "p (r m) -> p r m", r=NC),
                        ag[:, 1].rearrange("r p h t -> p r (h t)"))
                    # K^T for all heads in one DMA: [128, r, h*256]
                    kts = ktspool.tile([128, NC, H, SL], bf, tag="kts")
                    nc.sync.dma_start(
                        kts[:].rearrange("p r h t -> p r (h t)"),
                        ag[:, 0].rearrange("r p h t -> p r (h t)"))
                    own = vloc_tile is not None
                    npos = 2 * NC + (2 if own else 0)
                    masked = bias_cols is not None
                    for h in range(H):
                        avx = pav.tile([128, 2 * SL], f32, tag="avp",
                                       name="avx")
                        avp = avx[:, 0:SL]
                        dnp = avx[0:1, SL:2 * SL]
                        pos = 0
                        for g in range(NC // 2):
                            scp = psc.tile([128, 4 * SL], f32, tag="scp")
                            for j in range(4):
                                r, sub = 2 * g + j // 2, j % 2
                                nc.tensor.matmul(
                                    scp[:, j * SL:(j + 1) * SL],
                                    kts[:, r, h, sub * 128:(sub + 1) * 128],
                                    qtile[:, h, :], start=True, stop=True)
                            at = atpool.tile([128, 4 * SL], bf, tag="at")
                            nc.scalar.activation(at[:], scp[:], AF.Exp,
                                                 scale=1.0 / 1024.0)
                            if masked:
                                nc.vector.tensor_tensor(
                                    at[:], at[:],
                                    bias_cols[:, g * 1024:(g + 1) * 1024],
                                    op=OP.mult)
                            for j in range(4):
                                r, sub = 2 * g + j // 2, j % 2
                                ats = at[:, j * SL:(j + 1) * SL]
                                nc.tensor.matmul(
                                    avp, vag[:, r * 2 + sub,
                                                h * 128:(h + 1) * 128],
                                    ats, start=(pos == 0),
                                    stop=(pos == npos - 1))
                                nc.tensor.matmul(
                                    dnp, ones_bf[:], ats,
                                    start=(pos == 0), stop=(pos == npos - 1))
                                pos += 1
                        if own:
                            scp = psc.tile([128, 2 * SL], f32, tag="scp")
                            for sub in range(2):
                                nc.tensor.matmul(
                                    scp[:, sub * SL:(sub + 1) * SL],
                                    ktloc_tile[:, h, sub * 128:(sub + 1) * 128],
                                    qtile[:, h, :], start=True, stop=True)
                            at = atpool.tile([128, 2 * SL], bf, tag="at")
                            nc.scalar.activation(at[:], scp[:], AF.Exp,
                                                 scale=1.0 / 1024.0)
                            nc.vector.tensor_tensor(at[:], at[:], trics[:],
                                                    op=OP.mult)
                            for sub in range(2):
                                ats = at[:, sub * SL:(sub + 1) * SL]
                                nc.tensor.matmul(
                                    avp, vloc_tile[:, sub,
                                                      h * 128:(h + 1) * 128],
                                    ats, start=(pos == 0),
                                    stop=(pos == npos - 1))
                                nc.tensor.matmul(
                                    dnp, ones_bf[:], ats,
                                    start=(pos == 0), stop=(pos == npos - 1))
                                pos += 1
                        rc = atpool.tile([1, SL], f32, tag="rc")
                        nc.vector.reciprocal(rc[:], dnp)
                        rcb = prc.tile([128, SL], f32, tag="bcast", name="rcb")
                        nc.tensor.matmul(rcb[:], ones_row[:],
                                         rc[:], start=True,
                                         stop=True)
                        rcb_sb = atpool.tile([128, SL], f32, tag="rcb_sb")
                        nc.vector.tensor_copy(rcb_sb[:], rcb[:])
                        t1 = atpool.tile([128, SL], f32, tag="t1")
                        nc.vector.tensor_mul(t1[:], avp, rcb_sb[:])
                        nc.vector.scalar_tensor_tensor(
                            dst_pre[:, h, :], t1[:], bv_s[:, h:h + 1],
                            resid[:, h, :], op0=OP.add, op1=OP.add)

                # ---- self attention ----
                if phase_limit >= 2:
                    h1pre = mpool.tile([128, 8, SL], f32, tag="hpre",
                                       name="h1pre")
                    attention(ag_s, qts, vloc, ktloc, maskSs, bv1s, x32, h1pre)
                    h1 = mpool.tile([128, 8, SL], f32, tag="h", name="h1")
                    h1bf = mpool.tile([128, 8, SL], bf, tag="hbf", name="h1bf")
                    layer_norm(psc, prc, lnpool, h1pre, h1, h1bf)

                if phase_limit >= 3:
                    # ---- cross Q projection (stream wq2 now) ----
                    with tc.tile_pool(name="wq2p", bufs=1) as wq2pool:
                        wq2s = wq2pool.tile([128, 8, H * DKV], bf, name="wq2s")
                        nc.sync.dma_start(wq2s[:],
                                          wq2.rearrange("(c p) n -> p c n",
                                                        p=128))
                        q2ts = mpool.tile([128, H, SL], bf, tag="qt",
                                          name="q2ts")
                        for h in range(H):
                            ps = psc.tile([128, SL], f32, tag="scp")
                            for dc in range(8):
                                nc.tensor.matmul(
                                    ps[:], wq2s[:, dc, h * 128:(h + 1) * 128],
                                    h1bf[:, dc, :], start=(dc == 0),
                                    stop=(dc == 7))
                            nc.vector.tensor_scalar(q2ts[:, h, :], ps[:],
                                                    bq2s[:, h:h + 1], None,
                                                    op0=OP.add)

                    # ---- cross attention ----
                    h2pre = mpool.tile([128, 8, SL], f32, tag="hpre",
                                       name="h2pre")
                    attention(ag_c, q2ts, None, None, None, bv2s, h1, h2pre)
                    h2 = mpool.tile([128, 8, SL], f32, tag="h", name="h2")
                    h2bf = mpool.tile([128, 8, SL], bf, tag="hbf", name="h2bf")
                    layer_norm(psc, prc, lnpool, h2pre, h2, h2bf)

            # =========== FFN ===========
            if phase_limit < 4:
                nc.sync.dma_start(outT.rearrange("(c p) s -> p c s", p=128),
                                  x32[:])
                _done = True
            else:
                _done = False
            zbf = None
            if not _done:
                zbf = mpool.tile([128, FF // 128, SL], bf, name="zbf")
            if not _done:
              with tc.tile_pool(name="w1p", bufs=3) as w1pool, \
                 tc.tile_pool(name="pz", bufs=1, space="PSUM") as pz:
                for g in range(4):
                    zps = [pz.tile([128, SL], f32, tag=f"zps{ff}",
                                   name=f"zps{g}_{ff}")
                           for ff in range(8)]
                    w1t = w1pool.tile([128, 8, 1024], bf, tag="w1t")
                    nc.sync.dma_start(
                        w1t[:],
                        wf1[:, g * 1024:(g + 1) * 1024].rearrange(
                            "(c p) n -> p c n", p=128))
                    for dc in range(8):
                        for ff in range(8):
                            nc.tensor.matmul(zps[ff][:],
                                             w1t[:, dc,
                                                 ff * 128:(ff + 1) * 128],
                                             h2bf[:, dc, :], start=(dc == 0),
                                             stop=(dc == 7))
                    for ff in range(8):
                        fi = g * 8 + ff
                        nc.vector.tensor_scalar(zbf[:, fi, :], zps[ff][:],
                                                bf1s[:, fi:fi + 1], 0.0,
                                                op0=OP.add, op1=OP.max)

              h3pre = mpool.tile([128, 8, SL], f32, tag="hpre", name="h3pre")
              with tc.tile_pool(name="w2p", bufs=3) as w2pool, \
                 tc.tile_pool(name="po", bufs=1, space="PSUM") as po:
                ops = [po.tile([128, SL], f32, tag=f"ops{dc}",
                               name=f"ops{dc}")
                       for dc in range(8)]
                for fq in range(8):
                    w2t = w2pool.tile([128, 4, 1024], bf, tag="w2t")
                    nc.sync.dma_start(
                        w2t[:],
                        wf2[fq * 512:(fq + 1) * 512, :].rearrange(
                            "(c p) n -> p c n", p=128))
                    for fsub in range(4):
                        ffc = fq * 4 + fsub
                        for dc in range(8):
                            nc.tensor.matmul(ops[dc][:],
                                             w2t[:, fsub,
                                                 dc * 128:(dc + 1) * 128],
                                             zbf[:, ffc, :], start=(ffc == 0),
                                             stop=(ffc == 31))
                for dc in range(8):
                    nc.vector.scalar_tensor_tensor(
                        h3pre[:, dc, :], ops[dc][:], bf2s[:, dc:dc + 1],
                        h2[:, dc, :], op0=OP.add, op1=OP.add)

              out32 = mpool.tile([128, 8, SL], f32, tag="h", name="out32")
              with tc.tile_pool(name="pln3a", bufs=2, space="PSUM") as pa3, \
                 tc.tile_pool(name="pln3b", bufs=2, space="PSUM") as pb3, \
                 tc.tile_pool(name="ln3t", bufs=2) as lt3:
                layer_norm(pa3, pa3, lt3, h3pre, out32, None)

              nc.sync.dma_start(outT.rearrange("(c p) s -> p c s", p=128),
                                out32[:])

    nc.compile()
    return nc


def _get_program():
    if "nc" not in _prog_cache:
        _prog_cache["nc"] = _build_program()
    return _prog_cache["nc"]


def _prep_inputs(inputs):
    i = 5  # only the last layer matters (see module docstring)
    f32 = np.float32

    x = np.asarray(inputs["decoderInput"], f32)
    e = np.asarray(inputs["encoderOutput"], f32)

    def wcat(w):  # [H, D, dk] -> [D, H*dk]
        w = np.asarray(w, f32)
        return np.ascontiguousarray(w.transpose(1, 0, 2).reshape(D, H * DKV))

    shared = {
        "wq1": wcat(inputs["Wq1"][i]).astype(BF16),
        "wk1": wcat(inputs["Wk1"][i]).astype(BF16),
        "wv1": wcat(inputs["Wv1"][i]).astype(BF16),
        "wq2": wcat(inputs["Wq2"][i]).astype(BF16),
        "wk2": wcat(inputs["Wk2"][i]).astype(BF16),
        "wv2": wcat(inputs["Wv2"][i]).astype(BF16),
        "wf1": np.ascontiguousarray(np.asarray(inputs["Wff1"][i], f32)).astype(BF16),
        "wf2": np.ascontiguousarray(np.asarray(inputs["Wff2"][i], f32)).astype(BF16),
    }
    tt, ss = np.arange(128)[:, None], np.arange(SL)[None, :]
    tric = np.concatenate([(ss >= tt), (ss >= 128 + tt)], axis=1)
    shared["tric"] = tric.astype(BF16)

    bcommon = np.zeros((128, 112), f32)
    bcommon[:, 0:8] = np.asarray(inputs["bq1"][i], f32).T
    bcommon[:, 8:16] = np.asarray(inputs["bk1"][i], f32).T
    bcommon[:, 16:24] = np.asarray(inputs["bv1"][i], f32).T
    bcommon[:, 24:32] = np.asarray(inputs["bq2"][i], f32).T
    bcommon[:, 32:40] = np.asarray(inputs["bk2"][i], f32).T
    bcommon[:, 40:48] = np.asarray(inputs["bv2"][i], f32).T
    bcommon[:, 48:80] = np.asarray(inputs["bff1"][i], f32).reshape(FF // 128, 128).T
    bcommon[:, 80:88] = np.asarray(inputs["bff2"][i], f32).reshape(D // 128, 128).T
    bcommon[:, 88:96] = np.asarray(inputs["gamma"], f32).reshape(D // 128, 128).T
    bcommon[:, 96:104] = np.asarray(inputs["beta"], f32).reshape(D // 128, 128).T

    in_maps = []
    for c in range(NC):
        xc = x[c * SL:(c + 1) * SL, :].T  # [D, SL]
        ec = e[c * SL:(c + 1) * SL, :].T
        mask = np.zeros((128, 4096), np.float32)
        for r in range(NC):
            if r < c:  # strictly-past ranks fully visible in the AG path
                g, half = r // 2, r % 2
                mask[:, g * 1024 + half * 512: g * 1024 + (half + 1) * 512] = 1.0
        m = dict(shared)
        m["xT_bf"] = np.ascontiguousarray(xc).astype(BF16)
        m["xT_f32"] = np.ascontiguousarray(xc)
        m["eT_bf"] = np.ascontiguousarray(ec).astype(BF16)
        m["ball"] = bcommon
        m["maskS"] = mask.astype(BF16)
        in_maps.append(m)
    return in_maps


def _run(inputs, trace=False):
    from concourse.bass_utils import run_bass_kernel_spmd

    nc = _get_program()
    in_maps = _prep_inputs(inputs)
    res = run_bass_kernel_spmd(nc, in_maps, core_ids=list(range(NC)),
                               trace=trace)
    out = np.concatenate(
        [np.asarray(res.results[c]["outT"], np.float32).T for c in range(NC)],
        axis=0)
    return out, res


def kernel(**inputs) -> np.ndarray:
    out, _ = _run(inputs, trace=False)
    return out
